# revision 1
# baseline (speedup 1.0000x reference)
"""GCN message-passing kernel for 8 Trainium2 NeuronCores (Bass/Tile).

Algorithm (per core, nodes partitioned across cores):
  h0 = x @ W_enc + b_enc                      (local nodes)
  per conv layer l:
    hsW = (ds * h) @ W_conv[l]                (ds = 1/sqrt(deg), local)
    AllGather(hsW) -> hsW_full in HBM         (node-major, one row per node)
    agg[d] = sum over in-edges of hsW_full[src]   (bulk dma_gather + selection-
                                               matrix matmul reduce on PE)
    h = silu(ds * (agg + hsW) + b_conv[l])    (self-loop folds into hsW term)
  pooled = segment_sum(h, batch)              (selection-matrix matmul, no gather)
  AllReduce(pooled); out = relu(pooled @ W_out + b_out)

Host-side prep relabels nodes (pooling is order-invariant), partitions edges by
destination core, groups them by (src-window, dst-block) and pads to 128-edge
chunks.  Gather indices are int16 within a 25088-row window.  The per-chunk
reduce is out[d] += sum_e M[e, d] * msg[e] with M built on-chip by is_equal
against an iota row, so any per-chunk edge order/padding works.
"""

import numpy as np

# ---------------------------------------------------------------- config

CFG = dict(
    N=100000,          # nodes
    E=1600000,         # edges
    H=64,              # hidden
    L=4,               # conv layers
    G=512,             # graphs
    CORES=8,
    P=128,
    NB=98,             # node blocks per core (NLOC = 128*NB)
    NWIN=4,            # gather source windows (NTOT/NWIN <= 32767 for int16)
    CALL_CHUNKS=32,    # max 128-edge chunks per dma_gather call
)


def _derived(cfg):
    P, NB, CORES, NWIN = cfg["P"], cfg["NB"], cfg["CORES"], cfg["NWIN"]
    NLOC = P * NB
    NTOT = NLOC * CORES
    assert NTOT % NWIN == 0
    WIN = NTOT // NWIN
    assert WIN <= 32767, "int16 gather index range"
    GB = (cfg["G"] + P - 1) // P  # pool blocks
    assert cfg["G"] % P == 0
    return NLOC, NTOT, WIN, GB


# ---------------------------------------------------------------- host prep


def host_prep(x, edge_index, batch, cfg=None):
    """Permute nodes, build per-core inputs + gather schedule."""
    cfg = cfg or CFG
    N, E, H, CORES, P, NB, NWIN = (cfg[k] for k in
                                   ("N", "E", "H", "CORES", "P", "NB", "NWIN"))
    NLOC, NTOT, WIN, GB = _derived(cfg)
    CALL_CHUNKS = cfg["CALL_CHUNKS"]

    x = np.asarray(x, dtype=np.float32)
    edge_index = np.asarray(edge_index, dtype=np.int64)
    batch = np.asarray(batch, dtype=np.int64)
    src, dst = edge_index[0], edge_index[1]

    # degree (incoming + self loop) and 1/sqrt(deg)
    deg = np.bincount(dst, minlength=N).astype(np.float64) + 1.0
    ds = (1.0 / np.sqrt(deg)).astype(np.float32)

    # node -> global slot.  Each core gets N//CORES real nodes + dummies.
    per_core = N // CORES
    assert per_core * CORES == N and per_core <= NLOC
    rng = np.random.default_rng(12345)
    order = rng.permutation(N)  # order[j] = node at compact position j
    node_slot = np.empty(N, dtype=np.int64)
    for c in range(CORES):
        nodes_c = order[c * per_core:(c + 1) * per_core]
        node_slot[nodes_c] = c * NLOC + np.arange(per_core)

    # per-core node-major arrays (row j of core c = slot c*NLOC+j)
    xs, dss, bats = [], [], []
    for c in range(CORES):
        nodes_c = order[c * per_core:(c + 1) * per_core]
        xl = np.zeros((NLOC, H), dtype=np.float32)
        xl[:per_core] = x[nodes_c]
        dl = np.ones((NLOC,), dtype=np.float32)
        dl[:per_core] = ds[nodes_c]
        bl = np.full((NLOC,), -1.0, dtype=np.float32)
        bl[:per_core] = batch[nodes_c].astype(np.float32)
        xs.append(xl)
        # [P, NB] with position (d, b) = local slot d*NB+b
        dss.append(dl.reshape(P, NB).copy())
        bats.append(bl.reshape(P, NB).copy())

    # edges -> (core, window, block, within-window idx, dst partition slot)
    s_slot = node_slot[src]
    t_slot = node_slot[dst]
    e_core = t_slot // NLOC
    e_w = s_slot // WIN
    e_idx = (s_slot % WIN).astype(np.int64)
    t_loc = t_slot % NLOC
    e_d = t_loc // NB          # dst partition slot (0..127)
    e_b = t_loc % NB           # dst block

    # group counts per (core, w, b)
    key = (e_core * NWIN + e_w) * NB + e_b
    counts = np.bincount(key, minlength=CORES * NWIN * NB).reshape(CORES, NWIN, NB)
    chunks_wb = np.ceil(counts.max(axis=0) / P).astype(np.int64)  # [NWIN, NB]
    chunks_wb = np.maximum(chunks_wb, 0)

    # schedule: per window, calls = [(b, nchunks), ...] packed to CALL_CHUNKS
    schedule = []  # [w] -> list of calls; call = list of (b, nchunks)
    for w in range(NWIN):
        calls, cur, acc = [], [], 0
        for b in range(NB):
            nch = int(chunks_wb[w, b])
            if nch == 0:
                continue
            if acc + nch > CALL_CHUNKS and cur:
                calls.append(cur)
                cur, acc = [], 0
            cur.append((b, nch))
            acc += nch
        if cur:
            calls.append(cur)
        schedule.append(calls)

    total_chunks = int(chunks_wb.sum())
    total_idx = total_chunks * P

    # per-core gather index image + dst-rel image, in schedule order
    esort = np.lexsort((e_idx, e_b, e_w, e_core))
    s_sorted = esort  # edge ids sorted by (core, w, b, idx)
    e_idx_s = e_idx[s_sorted]
    e_d_s = e_d[s_sorted]
    # offsets of each (core, w, b) group in sorted array
    grp_key = key[s_sorted]
    grp_starts = np.searchsorted(grp_key, np.arange(CORES * NWIN * NB))
    grp_ends = np.searchsorted(grp_key, np.arange(CORES * NWIN * NB), side="right")

    idx_imgs, dstrel_imgs = [], []
    for c in range(CORES):
        flat_idx = np.zeros((total_idx,), dtype=np.int16)
        flat_d = np.full((total_chunks, P), -1.0, dtype=np.float32)
        pos = 0  # in chunks
        for w in range(NWIN):
            for call in schedule[w]:
                for b, nch in call:
                    g = (c * NWIN + w) * NB + b
                    s0, s1 = grp_starts[g], grp_ends[g]
                    n = s1 - s0
                    assert n <= nch * P
                    flat_idx[pos * P: pos * P + n] = e_idx_s[s0:s1]
                    fd = flat_d[pos: pos + nch].reshape(-1)
                    fd[:n] = e_d_s[s0:s1]
                    pos += nch
        assert pos == total_chunks
        # idx image: position t -> (t%16, t//16), replicated over 8 groups
        img = flat_idx.reshape(total_idx // 16, 16).T  # [16, total_idx//16]
        img = np.tile(img, (P // 16, 1)).copy()        # [128, total_idx//16]
        idx_imgs.append(img)
        # dstrel image: [P, total_chunks], (p, chunk) = dst slot or -1
        dstrel_imgs.append(np.ascontiguousarray(flat_d.T))

    return dict(
        xs=xs, dss=dss, bats=bats, idx_imgs=idx_imgs, dstrel_imgs=dstrel_imgs,
        schedule=schedule, total_chunks=total_chunks, total_idx=total_idx,
    )


# ---------------------------------------------------------------- bass build


def build_bass(schedule, total_chunks, total_idx, cfg=None):
    import concourse.bacc as bacc
    import concourse.tile as tile
    from concourse import mybir
    from concourse.masks import make_identity

    cfg = cfg or CFG
    N, E, H, L, G, CORES, P, NB, NWIN = (cfg[k] for k in
                                         ("N", "E", "H", "L", "G", "CORES",
                                          "P", "NB", "NWIN"))
    NLOC, NTOT, WIN, GB = _derived(cfg)
    f32, bf16, i16 = mybir.dt.float32, mybir.dt.bfloat16, mybir.dt.int16
    AF = mybir.ActivationFunctionType
    OP = mybir.AluOpType

    from concourse.bass import AP

    def mid_bcast(ap2d, nmid):
        # [P, X] -> [P, nmid, X] with middle step-0 broadcast
        return AP(ap2d.tensor, ap2d.offset,
                  [list(ap2d.ap[0]), [0, nmid], list(ap2d.ap[1])])

    nc = bacc.Bacc("TRN2", target_bir_lowering=False, debug=False,
                   num_devices=CORES)
    groups = [list(range(CORES))]

    # ---- I/O
    x_d = nc.dram_tensor("x", [NLOC, H], f32, kind="ExternalInput").ap()
    ds_d = nc.dram_tensor("ds", [P, NB], f32, kind="ExternalInput").ap()
    bat_d = nc.dram_tensor("bat", [P, NB], f32, kind="ExternalInput").ap()
    idx_d = nc.dram_tensor("idx", [P, total_idx // 16], i16,
                           kind="ExternalInput").ap()
    dstrel_d = nc.dram_tensor("dstrel", [P, total_chunks], f32,
                              kind="ExternalInput").ap()
    wenc_d = nc.dram_tensor("W_enc", [H, H], f32, kind="ExternalInput").ap()
    benc_d = nc.dram_tensor("b_enc", [P, H], f32, kind="ExternalInput").ap()
    wconv_d = nc.dram_tensor("W_conv", [L, H, H], f32, kind="ExternalInput").ap()
    bconv_d = nc.dram_tensor("b_conv", [P, L * H], f32, kind="ExternalInput").ap()
    wout_d = nc.dram_tensor("W_out", [H, 16], f32, kind="ExternalInput").ap()
    bout_d = nc.dram_tensor("b_out", [P, 16], f32, kind="ExternalInput").ap()
    iota_d = nc.dram_tensor("iota", [P, P], f32, kind="ExternalInput").ap()
    out_d = nc.dram_tensor("out", [G, 16], f32, kind="ExternalOutput").ap()

    # ---- internal DRAM
    agin = nc.dram_tensor("agin", [NLOC, H], f32).ap()
    hsw_full = nc.dram_tensor("hsw_full", [NTOT, H], f32).ap()
    arin = nc.dram_tensor("arin", [G, H], f32).ap()
    arout = nc.dram_tensor("arout", [G, H], f32).ap()

    with tile.TileContext(nc) as tc:
        with (
            tc.tile_pool(name="const", bufs=1) as cp,
            tc.tile_pool(name="state", bufs=1) as sp,
            tc.tile_pool(name="msg", bufs=4) as mp,
            tc.tile_pool(name="mb", bufs=2) as mbp,
            tc.tile_pool(name="lhs", bufs=3) as lp,
            tc.tile_pool(name="pa", bufs=3, space="PSUM") as pa,
            tc.tile_pool(name="pt", bufs=2, space="PSUM") as pt,
            tc.tile_pool(name="pm", bufs=2, space="PSUM") as pm,
        ):
            # ---- constants to SBUF
            ident = cp.tile([P, P], f32)
            make_identity(nc, ident[:])
            iota_t = cp.tile([P, P], f32)
            nc.sync.dma_start(out=iota_t[:], in_=iota_d[:])
            wenc_t = cp.tile([H, H], f32)
            nc.sync.dma_start(out=wenc_t[:], in_=wenc_d[:])
            benc_t = cp.tile([P, H], f32)
            nc.sync.dma_start(out=benc_t[:], in_=benc_d[:])
            wconv_t = cp.tile([H, L * H], f32)
            nc.sync.dma_start(
                out=wconv_t[:].rearrange("i (l o) -> i l o", l=L),
                in_=wconv_d[:].rearrange("l i o -> i l o"))
            bconv_t = cp.tile([P, L * H], f32)
            nc.sync.dma_start(out=bconv_t[:], in_=bconv_d[:])
            wout_t = cp.tile([H, 16], f32)
            nc.sync.dma_start(out=wout_t[:], in_=wout_d[:])
            bout_t = cp.tile([P, 16], f32)
            nc.sync.dma_start(out=bout_t[:], in_=bout_d[:])
            ds_t = cp.tile([P, NB], f32)
            nc.sync.dma_start(out=ds_t[:], in_=ds_d[:])
            bat_t = cp.tile([P, NB], f32)
            nc.sync.dma_start(out=bat_t[:], in_=bat_d[:])
            idx_t = cp.tile([P, total_idx // 16], i16)
            nc.sync.dma_start(out=idx_t[:], in_=idx_d[:])
            dstrel_t = cp.tile([P, total_chunks], f32)
            nc.sync.dma_start(out=dstrel_t[:], in_=dstrel_d[:])

            # ---- state tiles (layout [P(d), NB, H], local node = d*NB+b)
            h_t = sp.tile([P, NB * H], f32, tag="h")
            hsw_t = sp.tile([P, NB * H], f32, tag="hsw")
            agg_t = sp.tile([P, NB * H], f32, tag="agg")
            h3 = h_t[:].rearrange("p (b f) -> p b f", b=NB)
            hsw3 = hsw_t[:].rearrange("p (b f) -> p b f", b=NB)
            agg3 = agg_t[:].rearrange("p (b f) -> p b f", b=NB)

            def block_mm(src3, b, w_ap, psum_out):
                """psum_out[P, H] = src3[:, b, :] @ w_ap  (via PE transpose)."""
                ptile = pt.tile([H, P], f32, tag="ptr")
                nc.tensor.transpose(out=ptile[:], in_=src3[:, b, :],
                                    identity=ident[:])
                lhs = lp.tile([H, P], f32, tag="lhs")
                nc.vector.tensor_copy(out=lhs[:], in_=ptile[:])
                nc.tensor.matmul(out=psum_out, lhsT=lhs[:], rhs=w_ap,
                                 start=True, stop=True)

            # ---- prelude: h0 = x @ W_enc + b_enc
            nc.sync.dma_start(
                out=agg_t[:],
                in_=x_d[:].rearrange("(d b) f -> d (b f)", d=P))
            x3 = agg3
            for b in range(NB):
                pmm = pm.tile([P, H], f32, tag="pmm")
                block_mm(x3, b, wenc_t[:], pmm[:])
                nc.vector.tensor_tensor(
                    out=h3[:, b, :], in0=pmm[:], in1=benc_t[:],
                    op=OP.add)

            # ---- conv layers
            for l in range(L):
                # hs = h * ds  (in place)
                nc.vector.tensor_tensor(
                    out=h3, in0=h3,
                    in1=ds_t[:].to_broadcast([P, NB, H]),
                    op=OP.mult)
                # hsW = hs @ W_conv[l]
                wl = wconv_t[:].rearrange("i (l o) -> l i o", l=L)[l]
                for b in range(NB):
                    pmm = pm.tile([P, H], f32, tag="pmm")
                    block_mm(h3, b, wl, pmm[:])
                    nc.vector.tensor_copy(out=hsw3[:, b, :], in_=pmm[:])
                # ship local hsW, AllGather into hsw_full
                nc.sync.dma_start(
                    out=agin[:].rearrange("(d b) f -> d (b f)", d=P),
                    in_=hsw_t[:])
                nc.gpsimd.collective_compute(
                    "AllGather", OP.bypass, replica_groups=groups,
                    ins=[agin[:]], outs=[hsw_full[:]])

                # aggregate
                nc.vector.memset(agg_t[:], 0.0)
                chunk_off = 0
                for w in range(NWIN):
                    win_ap = hsw_full[w * WIN:(w + 1) * WIN, :]
                    for call in schedule[w]:
                        nch = sum(n for _, n in call)
                        nidx = nch * P
                        msg = mp.tile([P, cfg["CALL_CHUNKS"] * H], f32,
                                      tag="msg")
                        nc.gpsimd.dma_gather(
                            out_ap=msg[:, :nch * H]
                                .rearrange("p (s f) -> p s f", f=H),
                            in_ap=win_ap,
                            idxs_ap=idx_t[:, chunk_off * 8:
                                          chunk_off * 8 + nidx // 16],
                            num_idxs=nidx,
                            num_idxs_reg=nidx,
                            elem_size=H,
                            single_packet=False,
                        )
                        mtile = mbp.tile([P, cfg["CALL_CHUNKS"] * P], f32,
                                         tag="mtile")
                        nc.vector.tensor_tensor(
                            out=mtile[:, :nch * P]
                                .rearrange("p (c q) -> p c q", q=P),
                            in0=dstrel_t[:, chunk_off:chunk_off + nch]
                                .to_broadcast([P, nch, P]),
                            in1=mid_bcast(iota_t[:], nch),
                            op=OP.is_equal)
                        msgb3 = msg[:].rearrange("p (s f) -> p s f", f=H)
                        mt3 = mtile[:].rearrange("p (c q) -> p c q", q=P)
                        ck = 0
                        for b, nch_b in call:
                            pagg = pa.tile([P, H], f32, tag="pagg")
                            for k in range(nch_b):
                                nc.tensor.matmul(
                                    out=pagg[:], lhsT=mt3[:, ck + k, :],
                                    rhs=msgb3[:, ck + k, :],
                                    start=(k == 0), stop=(k == nch_b - 1))
                            ck += nch_b
                            nc.vector.tensor_tensor(
                                out=agg3[:, b, :], in0=agg3[:, b, :],
                                in1=pagg[:], op=OP.add)
                        chunk_off += nch
                assert w == NWIN - 1

                # h = silu(ds * (agg + hsW) + b_conv[l])
                nc.vector.tensor_tensor(out=agg_t[:], in0=agg_t[:],
                                        in1=hsw_t[:], op=OP.add)
                nc.vector.tensor_tensor(
                    out=agg3, in0=agg3,
                    in1=ds_t[:].to_broadcast([P, NB, H]),
                    op=OP.mult)
                nc.vector.tensor_tensor(
                    out=agg3, in0=agg3,
                    in1=mid_bcast(bconv_t[:, l * H:(l + 1) * H], NB),
                    op=OP.add)
                # silu(x) = x * sigmoid(x) (two-op: CoreSim lacks Silu)
                nc.scalar.activation(out=h_t[:], in_=agg_t[:], func=AF.Sigmoid)
                nc.vector.tensor_tensor(out=h_t[:], in0=h_t[:], in1=agg_t[:],
                                        op=OP.mult)

            # ---- pooling: pooled[pb*P+g] = sum_{nodes with bat==pb*P+g} h
            pooled = sp.tile([P, GB * H], f32, tag="pooled")
            MG = 14  # blocks per Mp build
            for pb in range(GB):
                ppool = pa.tile([P, H], f32, tag="pagg")
                for b0 in range(0, NB, MG):
                    nbk = min(MG, NB - b0)
                    mg = mbp.tile([P, MG * P], f32, tag="mg")
                    # mg[d, (b, g)] = (bat[d, b0+b] - pb*P == iota[g])
                    nc.vector.tensor_scalar(
                        out=mg[:, :nbk * P]
                            .rearrange("p (c q) -> p c q", q=P),
                        in0=bat_t[:, b0:b0 + nbk]
                            .to_broadcast([P, nbk, P]),
                        scalar1=float(pb * P),
                        scalar2=None,
                        op0=OP.subtract,
                    )
                    nc.vector.tensor_tensor(
                        out=mg[:, :nbk * P]
                            .rearrange("p (c q) -> p c q", q=P),
                        in0=mg[:, :nbk * P]
                            .rearrange("p (c q) -> p c q", q=P),
                        in1=mid_bcast(iota_t[:], nbk),
                        op=OP.is_equal)
                    mg3 = mg[:].rearrange("p (c q) -> p c q", q=P)
                    for j in range(nbk):
                        b = b0 + j
                        nc.tensor.matmul(
                            out=ppool[:], lhsT=mg3[:, j, :], rhs=h3[:, b, :],
                            start=(b == 0), stop=(b == NB - 1))
                nc.vector.tensor_copy(
                    out=pooled[:].rearrange("p (pb f) -> p pb f", pb=GB)[:, pb, :],
                    in_=ppool[:])
            nc.sync.dma_start(
                out=arin[:].rearrange("(pb g) f -> g pb f", pb=GB),
                in_=pooled[:].rearrange("p (pb f) -> p pb f", pb=GB))
            nc.gpsimd.collective_compute(
                "AllReduce", OP.add, replica_groups=groups,
                ins=[arin[:]], outs=[arout[:]])
            pooled_f = sp.tile([P, GB * H], f32, tag="pooledf")
            nc.sync.dma_start(
                out=pooled_f[:].rearrange("p (pb f) -> p pb f", pb=GB),
                in_=arout[:].rearrange("(pb g) f -> g pb f", pb=GB))
            pf3 = pooled_f[:].rearrange("p (pb f) -> p pb f", pb=GB)

            # ---- readout
            outs = sp.tile([P, GB * 16], f32, tag="outs")
            o3 = outs[:].rearrange("p (pb t) -> p pb t", pb=GB)
            for pb in range(GB):
                pmm = pm.tile([P, 16], f32, tag="pmm")
                block_mm(pf3, pb, wout_t[:], pmm[:])
                nc.vector.tensor_tensor(
                    out=pmm[:], in0=pmm[:], in1=bout_t[:], op=OP.add)
                nc.scalar.activation(out=o3[:, pb, :], in_=pmm[:],
                                     func=AF.Relu)
            nc.sync.dma_start(
                out=out_d[:].rearrange("(pb g) t -> g pb t", pb=GB),
                in_=outs[:].rearrange("p (pb t) -> p pb t", pb=GB))

    nc.compile()
    return nc


# ---------------------------------------------------------------- entry

_CACHE = {}
TRACE = False
LAST_RESULTS = None


def kernel(x, edge_index, batch, W_enc, b_enc, W_conv, b_conv, W_out, b_out,
           num_graphs):
    cfg = CFG
    N, E, H, L, G, CORES = (cfg[k] for k in ("N", "E", "H", "L", "G", "CORES"))
    P = cfg["P"]
    NLOC, NTOT, WIN, GB = _derived(cfg)

    prep = host_prep(x, edge_index, batch, cfg)
    key = (prep["total_chunks"],
           tuple(tuple(tuple(c) for call in wcalls for c in call)
                 for wcalls in prep["schedule"]))
    if key not in _CACHE:
        from concourse import bass_utils  # noqa: F401
        _CACHE.clear()
        _CACHE[key] = build_bass(prep["schedule"], prep["total_chunks"],
                                 prep["total_idx"], cfg)
    nc = _CACHE[key]

    W_out16 = np.zeros((H, 16), dtype=np.float32)
    W_out16[:, :10] = np.asarray(W_out, dtype=np.float32)
    b_out16 = np.zeros((1, 16), dtype=np.float32)
    b_out16[0, :10] = np.asarray(b_out, dtype=np.float32)

    shared = {
        "W_enc": np.asarray(W_enc, dtype=np.float32),
        "b_enc": np.tile(np.asarray(b_enc, dtype=np.float32).reshape(1, H), (P, 1)),
        "W_conv": np.asarray(W_conv, dtype=np.float32),
        "b_conv": np.tile(np.asarray(b_conv, dtype=np.float32).reshape(1, -1), (P, 1)),
        "W_out": W_out16,
        "b_out": np.tile(b_out16, (P, 1)),
        "iota": np.tile(np.arange(P, dtype=np.float32), (P, 1)),
    }
    in_maps = []
    for c in range(CORES):
        m = dict(shared)
        m["x"] = prep["xs"][c]
        m["ds"] = prep["dss"][c]
        m["bat"] = prep["bats"][c]
        m["idx"] = prep["idx_imgs"][c]
        m["dstrel"] = prep["dstrel_imgs"][c]
        in_maps.append(m)

    from concourse.bass_utils import run_bass_kernel_spmd
    res = run_bass_kernel_spmd(nc, in_maps, core_ids=list(range(CORES)),
                               trace=TRACE)
    global LAST_RESULTS
    LAST_RESULTS = res
    out = res.results[0]["out"]  # [G, 16]
    return np.ascontiguousarray(out[:, :10].astype(np.float32))



# revision 12
# speedup vs baseline: 1.9934x; 1.9934x over previous
"""GCN message-passing kernel for 8 Trainium2 NeuronCores (Bass/Tile).

v2: bf16 pair-gather on 4 SWDGE queues.

Algorithm (per core, nodes partitioned across cores):
  h0 = x @ W_enc + b_enc                      (fp32 encoder, bf16 state)
  per conv layer l:
    hsW = (ds * h) @ W_conv[l]                (bf16 blocks on PE)
    AllGather(hsW) -> hsw_full bf16 [PAIRS, 128] (pair-major, 256B rows)
    agg[d] = sum over in-edges of hsw_full[src]:
      dma_gather of PAIRS (256B bf16 descriptors, 4 SWDGE queues RR),
      per-chunk selection matmuls (even/odd half) reduce on PE into PSUM
    h = silu(ds * (agg + hsW) + b_conv[l])    (fp32 compute, bf16 store)
  pooled = segment_sum(h, batch)              (selection-matmul, bf16)
  AllReduce(pooled); out = relu(pooled @ W_out + b_out)

Host-side prep relabels nodes, partitions edges by destination core, groups
them by (src pair-window, dst block), one gather call per group.  Padding
slots gather pair 0 (harmless) and carry dstrel = -1.  Gather indices are
int16 PAIR offsets within a 25088-pair window.  The per-chunk reduce uses
M_even/M_odd selection matrices built on-chip by is_equal against an iota row
(int16 in, bf16 out), so padded slots (dstrel = -1) contribute zero.
"""

import numpy as np

# ---------------------------------------------------------------- config

CFG = dict(
    N=100000,          # nodes
    E=1600000,         # edges
    H=64,              # hidden
    L=4,               # conv layers
    G=512,             # graphs
    CORES=8,
    P=128,
    NB=98,             # node blocks per core (NLOC = 128*NB)
    NWIN=2,            # gather source windows in PAIRS (<= 32767 idx range)
    NQ=4,              # SWDGE queues
)


def _derived(cfg):
    P, NB, CORES, NWIN = cfg["P"], cfg["NB"], cfg["CORES"], cfg["NWIN"]
    NLOC = P * NB
    NTOT = NLOC * CORES
    PAIRS = NTOT // 2
    assert PAIRS % NWIN == 0
    WINP = PAIRS // NWIN
    assert WINP <= 32767, "int16 gather index range"
    GB = (cfg["G"] + P - 1) // P
    assert cfg["G"] % P == 0
    return NLOC, NTOT, PAIRS, WINP, GB


# ---------------------------------------------------------------- host prep


def host_prep(x, edge_index, batch, cfg=None):
    """Permute nodes, build per-core inputs + gather schedule."""
    cfg = cfg or CFG
    N, E, H, CORES, P, NB, NWIN = (cfg[k] for k in
                                   ("N", "E", "H", "CORES", "P", "NB", "NWIN"))
    NLOC, NTOT, PAIRS, WINP, GB = _derived(cfg)

    x = np.asarray(x, dtype=np.float32)
    edge_index = np.asarray(edge_index, dtype=np.int64)
    batch = np.asarray(batch, dtype=np.int64)
    src, dst = edge_index[0], edge_index[1]

    deg = np.bincount(dst, minlength=N).astype(np.float64) + 1.0
    ds = (1.0 / np.sqrt(deg)).astype(np.float32)

    # node -> global slot.  Each core gets N//CORES real nodes + dummies.
    per_core = N // CORES
    assert per_core * CORES == N and per_core <= NLOC
    rng = np.random.default_rng(12345)
    order = rng.permutation(N)
    node_slot = np.empty(N, dtype=np.int64)
    for c in range(CORES):
        nodes_c = order[c * per_core:(c + 1) * per_core]
        node_slot[nodes_c] = c * NLOC + np.arange(per_core)

    xs, dss, bats = [], [], []
    for c in range(CORES):
        nodes_c = order[c * per_core:(c + 1) * per_core]
        xl = np.zeros((NLOC, H), dtype=np.float32)
        xl[:per_core] = x[nodes_c]
        dl = np.ones((NLOC,), dtype=np.float32)
        dl[:per_core] = ds[nodes_c]
        bl = np.full((NLOC,), -1, dtype=np.int64)
        bl[:per_core] = batch[nodes_c]
        xs.append(xl)
        dss.append(dl.reshape(P, NB).copy())
        bats.append(bl.reshape(P, NB).astype(np.int16).copy())

    # edges -> (core, window, block, pair idx, parity, dst partition slot)
    s_slot = node_slot[src]
    t_slot = node_slot[dst]
    e_core = t_slot // NLOC
    s_pair = s_slot // 2
    e_par = (s_slot % 2).astype(np.int64)
    e_w = s_pair // WINP
    e_idx = (s_pair % WINP).astype(np.int64)
    t_loc = t_slot % NLOC
    e_d = t_loc // NB
    e_b = t_loc % NB

    key = (e_core * NWIN + e_w) * NB + e_b
    counts = np.bincount(key, minlength=CORES * NWIN * NB).reshape(CORES, NWIN, NB)
    chunks_wb = np.ceil(counts.max(axis=0) / P).astype(np.int64)  # [NWIN, NB]

    # schedule: per window, one call per (b) group
    schedule = []  # [w] -> [(b, nch), ...]
    for w in range(NWIN):
        calls = [(int(b), int(chunks_wb[w, b])) for b in range(NB)
                 if chunks_wb[w, b] > 0]
        schedule.append(calls)

    total_chunks = int(chunks_wb.sum())
    total_idx = total_chunks * P
    max_nch = int(chunks_wb.max())

    # per-core gather index image + dst-rel even/odd image, in schedule order
    esort = np.lexsort((e_idx, e_b, e_w, e_core))
    e_idx_s = e_idx[esort]
    e_d_s = e_d[esort]
    e_par_s = e_par[esort]
    grp_key = key[esort]
    grp_starts = np.searchsorted(grp_key, np.arange(CORES * NWIN * NB))
    grp_ends = np.searchsorted(grp_key, np.arange(CORES * NWIN * NB), side="right")

    idx_imgs, dstrel_imgs = [], []
    for c in range(CORES):
        flat_idx = np.zeros((total_idx,), dtype=np.int16)
        # [chunk, P, 2] -> (even dst, odd dst), -1 = unused
        flat_d = np.full((total_chunks, P, 2), -1, dtype=np.int16)
        pos = 0  # in chunks
        for w in range(NWIN):
            for b, nch in schedule[w]:
                g = (c * NWIN + w) * NB + b
                s0, s1 = grp_starts[g], grp_ends[g]
                n = s1 - s0
                assert n <= nch * P
                flat_idx[pos * P: pos * P + n] = e_idx_s[s0:s1]
                fd = flat_d[pos: pos + nch].reshape(-1, 2)
                rows = np.arange(n)
                fd[rows, e_par_s[s0:s1]] = e_d_s[s0:s1]
                pos += nch
        assert pos == total_chunks
        img = flat_idx.reshape(total_idx // 16, 16).T  # [16, total_idx//16]
        img = np.tile(img, (P // 16, 1)).copy()
        idx_imgs.append(img)
        # dstrel image: [P, total_chunks*2], (p, 2c+par) = dst slot or -1
        dstrel_imgs.append(
            np.ascontiguousarray(flat_d.transpose(1, 0, 2).reshape(P, -1)))

    return dict(
        xs=xs, dss=dss, bats=bats, idx_imgs=idx_imgs, dstrel_imgs=dstrel_imgs,
        schedule=schedule, total_chunks=total_chunks, total_idx=total_idx,
        max_nch=max_nch,
    )


# ---------------------------------------------------------------- bass build


def build_bass(schedule, total_chunks, total_idx, max_nch, cfg=None):
    import os
    DEBUG_MODE = os.environ.get("GCN_DEBUG", "")
    import concourse.bacc as bacc
    import concourse.tile as tile
    from concourse import mybir
    from concourse.masks import make_identity

    cfg = cfg or CFG
    N, E, H, L, G, CORES, P, NB, NWIN, NQ = (cfg[k] for k in
                                             ("N", "E", "H", "L", "G", "CORES",
                                              "P", "NB", "NWIN", "NQ"))
    NLOC, NTOT, PAIRS, WINP, GB = _derived(cfg)
    f32, bf16, i16 = mybir.dt.float32, mybir.dt.bfloat16, mybir.dt.int16
    AF = mybir.ActivationFunctionType
    OP = mybir.AluOpType

    from concourse.bass import AP

    def mid_bcast(ap2d, nmid):
        return AP(ap2d.tensor, ap2d.offset,
                  [list(ap2d.ap[0]), [0, nmid], list(ap2d.ap[1])])

    nc = bacc.Bacc("TRN2", target_bir_lowering=False, debug=False,
                   num_devices=CORES, num_swdge_queues=NQ)
    groups = [list(range(CORES))]

    # ---- I/O
    x_d = nc.dram_tensor("x", [NLOC, H], f32, kind="ExternalInput").ap()
    ds_d = nc.dram_tensor("ds", [P, NB], f32, kind="ExternalInput").ap()
    bat_d = nc.dram_tensor("bat", [P, NB], i16, kind="ExternalInput").ap()
    idx_d = nc.dram_tensor("idx", [P, total_idx // 16], i16,
                           kind="ExternalInput").ap()
    dstrel_d = nc.dram_tensor("dstrel", [P, 2 * total_chunks], i16,
                              kind="ExternalInput").ap()
    wenc_d = nc.dram_tensor("W_enc", [H, H], f32, kind="ExternalInput").ap()
    benc_d = nc.dram_tensor("b_enc", [P, H], f32, kind="ExternalInput").ap()
    wconv_d = nc.dram_tensor("W_conv", [H, L * H], bf16,
                             kind="ExternalInput").ap()
    bconv_d = nc.dram_tensor("b_conv", [P, L * H], f32, kind="ExternalInput").ap()
    wout_d = nc.dram_tensor("W_out", [H, 16], f32, kind="ExternalInput").ap()
    bout_d = nc.dram_tensor("b_out", [P, 16], f32, kind="ExternalInput").ap()
    iota16_d = nc.dram_tensor("iota16", [P, P], i16, kind="ExternalInput").ap()
    out_d = nc.dram_tensor("out", [G, 16], f32, kind="ExternalOutput").ap()

    # ---- internal DRAM (pair-major bf16 gather table)
    agin = nc.dram_tensor("agin", [NLOC // 2, P], bf16).ap()
    hsw_full = nc.dram_tensor("hsw_full", [PAIRS, P], bf16,
                              addr_space="Shared").ap()
    arin = nc.dram_tensor("arin", [G, H], f32).ap()
    arout = nc.dram_tensor("arout", [G, H], f32).ap()

    with tile.TileContext(nc) as tc:
        with (
            tc.tile_pool(name="const", bufs=1) as cp,
            tc.tile_pool(name="state", bufs=1) as sp,
            tc.tile_pool(name="msg", bufs=8) as mp,
            tc.tile_pool(name="mb", bufs=4) as mbp,
            tc.tile_pool(name="mgp", bufs=2) as mgp,
            tc.tile_pool(name="lhs", bufs=3) as lp,
            tc.tile_pool(name="pa", bufs=3, space="PSUM") as pa,
            tc.tile_pool(name="pt", bufs=1, space="PSUM") as pt,
            tc.tile_pool(name="ptb", bufs=1, space="PSUM") as ptb,
            tc.tile_pool(name="pm", bufs=2, space="PSUM") as pm,
        ):
            # ---- constants to SBUF
            ident = cp.tile([P, P], f32)
            make_identity(nc, ident[:])
            ident_bf = cp.tile([P, P], bf16)
            nc.vector.tensor_copy(out=ident_bf[:], in_=ident[:])
            iota16_t = cp.tile([P, P], i16)
            nc.sync.dma_start(out=iota16_t[:], in_=iota16_d[:])
            wenc_t = cp.tile([H, H], f32)
            nc.sync.dma_start(out=wenc_t[:], in_=wenc_d[:])
            benc_t = cp.tile([P, H], f32)
            nc.sync.dma_start(out=benc_t[:], in_=benc_d[:])
            wconv_t = cp.tile([H, L * H], bf16)
            nc.sync.dma_start(out=wconv_t[:], in_=wconv_d[:])
            bconv_t = cp.tile([P, L * H], f32)
            nc.sync.dma_start(out=bconv_t[:], in_=bconv_d[:])
            wout_t = cp.tile([H, 16], f32)
            nc.sync.dma_start(out=wout_t[:], in_=wout_d[:])
            bout_t = cp.tile([P, 16], f32)
            nc.sync.dma_start(out=bout_t[:], in_=bout_d[:])
            ds_t = cp.tile([P, NB], f32)
            nc.sync.dma_start(out=ds_t[:], in_=ds_d[:])
            bat_t = cp.tile([P, NB], i16)
            nc.sync.dma_start(out=bat_t[:], in_=bat_d[:])
            idx_t = cp.tile([P, total_idx // 16], i16)
            nc.sync.dma_start(out=idx_t[:], in_=idx_d[:])
            dstrel_t = cp.tile([P, 2 * total_chunks], i16)
            nc.sync.dma_start(out=dstrel_t[:], in_=dstrel_d[:])

            # ---- state tiles (layout [P(d), NB, H], local node = d*NB+b)
            h_t = sp.tile([P, NB * H], bf16, tag="h")
            hsw_t = sp.tile([P, NB * H], bf16, tag="hsw")
            agg_t = sp.tile([P, NB * H], f32, tag="agg")
            sig_t = sp.tile([P, NB * H], f32, tag="sig")
            h3 = h_t[:].rearrange("p (b f) -> p b f", b=NB)
            hsw3 = hsw_t[:].rearrange("p (b f) -> p b f", b=NB)
            agg3 = agg_t[:].rearrange("p (b f) -> p b f", b=NB)

            # ---- prelude: h0 = x @ W_enc + b_enc (fp32, store bf16)
            nc.sync.dma_start(
                out=agg_t[:],
                in_=x_d[:].rearrange("(d b) f -> d (b f)", d=P))
            x3 = agg3
            for b in range(NB):
                ptile = pt.tile([H, P], f32, tag="ptr")
                nc.tensor.transpose(out=ptile[:], in_=x3[:, b, :],
                                    identity=ident[:])
                lhs = lp.tile([H, P], f32, tag="lhsf")
                nc.vector.tensor_copy(out=lhs[:], in_=ptile[:])
                pmm = pm.tile([P, H], f32, tag="pmm")
                nc.tensor.matmul(out=pmm[:], lhsT=lhs[:], rhs=wenc_t[:],
                                 start=True, stop=True)
                nc.vector.tensor_tensor(
                    out=h3[:, b, :], in0=pmm[:], in1=benc_t[:], op=OP.add)

            # ---- conv layers
            for l in range(L):
                # hs = h * ds  (in place, bf16)
                nc.vector.tensor_tensor(
                    out=h3, in0=h3,
                    in1=ds_t[:].to_broadcast([P, NB, H]),
                    op=OP.mult)
                # hsW = hs @ W_conv[l]  (bf16)
                wl = wconv_t[:].rearrange("i (l o) -> l i o", l=L)[l]
                for b in range(NB):
                    ptile = ptb.tile([H, P], bf16, tag="ptrb")
                    nc.tensor.transpose(out=ptile[:], in_=h3[:, b, :],
                                        identity=ident_bf[:])
                    lhs = lp.tile([H, P], bf16, tag="lhsb")
                    nc.vector.tensor_copy(out=lhs[:], in_=ptile[:])
                    pmm = pm.tile([P, H], f32, tag="pmm")
                    nc.tensor.matmul(out=pmm[:], lhsT=lhs[:], rhs=wl,
                                     start=True, stop=True)
                    nc.vector.tensor_copy(out=hsw3[:, b, :], in_=pmm[:])
                # ship local hsW (pair-major), AllGather into hsw_full
                nc.sync.dma_start(
                    out=agin[:].rearrange("(d bb) tf -> d (bb tf)", d=P),
                    in_=hsw_t[:])
                nc.gpsimd.collective_compute(
                    "AllGather", OP.bypass, replica_groups=groups,
                    ins=[agin[:]], outs=[hsw_full[:]])

                # aggregate
                nc.vector.memset(agg_t[:], 0.0)
                chunk_off = 0
                call_no = 0
                for w in range(NWIN):
                    if DEBUG_MODE == "nogather":
                        continue
                    win_ap = hsw_full[w * WINP:(w + 1) * WINP, :]
                    for b, nch in schedule[w]:
                        msg = mp.tile([P, max_nch * P], bf16, tag="msg")
                        nc.gpsimd.dma_gather(
                            out_ap=msg[:, :nch * P]
                                .rearrange("p (s f) -> p s f", f=P),
                            in_ap=win_ap,
                            idxs_ap=idx_t[:, chunk_off * 8:
                                          chunk_off * 8 + nch * 8],
                            num_idxs=nch * P,
                            num_idxs_reg=nch * P,
                            elem_size=P,
                            single_packet=False,
                            queue_num=call_no % NQ,
                        )
                        if DEBUG_MODE == "gatheronly":
                            nc.vector.tensor_tensor(
                                out=agg3[:, b, :], in0=agg3[:, b, :],
                                in1=msg[:, 0:H], op=OP.add)
                            chunk_off += nch
                            call_no += 1
                            continue
                        mtile = mbp.tile([P, max_nch * 2 * P], bf16,
                                         tag="mtile")
                        nc.vector.tensor_tensor(
                            out=mtile[:, :nch * 2 * P]
                                .rearrange("p (c q) -> p c q", q=P),
                            in0=dstrel_t[:, 2 * chunk_off:
                                         2 * (chunk_off + nch)]
                                .to_broadcast([P, 2 * nch, P]),
                            in1=mid_bcast(iota16_t[:], 2 * nch),
                            op=OP.is_equal)
                        msg3 = msg[:].rearrange("p (s f) -> p s f", f=P)
                        mt3 = mtile[:].rearrange("p (c q) -> p c q", q=P)
                        pagg = pa.tile([P, H], f32, tag="pagg")
                        for k in range(nch):
                            nc.tensor.matmul(
                                out=pagg[:], lhsT=mt3[:, 2 * k, :],
                                rhs=msg3[:, k, 0:H],
                                start=(k == 0), stop=False)
                            nc.tensor.matmul(
                                out=pagg[:], lhsT=mt3[:, 2 * k + 1, :],
                                rhs=msg3[:, k, H:2 * H],
                                start=False, stop=(k == nch - 1))
                        nc.vector.tensor_tensor(
                            out=agg3[:, b, :], in0=agg3[:, b, :],
                            in1=pagg[:], op=OP.add)
                        chunk_off += nch
                        call_no += 1
                if not DEBUG_MODE:
                    assert chunk_off == total_chunks

                # h = silu(ds * (agg + hsW) + b_conv[l])
                nc.vector.tensor_tensor(out=agg_t[:], in0=agg_t[:],
                                        in1=hsw_t[:], op=OP.add)
                nc.vector.tensor_tensor(
                    out=agg3, in0=agg3,
                    in1=ds_t[:].to_broadcast([P, NB, H]),
                    op=OP.mult)
                nc.vector.tensor_tensor(
                    out=agg3, in0=agg3,
                    in1=mid_bcast(bconv_t[:, l * H:(l + 1) * H], NB),
                    op=OP.add)
                nc.scalar.activation(out=sig_t[:], in_=agg_t[:],
                                     func=AF.Sigmoid)
                nc.vector.tensor_tensor(out=h_t[:], in0=sig_t[:],
                                        in1=agg_t[:], op=OP.mult)

            # ---- pooling: pooled[pb*P+g] = sum_{nodes with bat==pb*P+g} h
            pooled = sp.tile([P, GB * H], f32, tag="pooled")
            MG = 14  # blocks per Mp build
            for pb in range(GB):
                ppool = pa.tile([P, H], f32, tag="pagg")
                for b0 in range(0, NB, MG):
                    nbk = min(MG, NB - b0)
                    mg = mgp.tile([P, MG * P], bf16, tag="mg")
                    mgi = mgp.tile([P, MG * P], i16, tag="mgi")
                    nc.vector.tensor_scalar(
                        out=mgi[:, :nbk * P]
                            .rearrange("p (c q) -> p c q", q=P),
                        in0=bat_t[:, b0:b0 + nbk]
                            .to_broadcast([P, nbk, P]),
                        scalar1=pb * P,
                        scalar2=None,
                        op0=OP.subtract,
                    )
                    nc.vector.tensor_tensor(
                        out=mg[:, :nbk * P]
                            .rearrange("p (c q) -> p c q", q=P),
                        in0=mgi[:, :nbk * P]
                            .rearrange("p (c q) -> p c q", q=P),
                        in1=mid_bcast(iota16_t[:], nbk),
                        op=OP.is_equal)
                    mg3 = mg[:].rearrange("p (c q) -> p c q", q=P)
                    for j in range(nbk):
                        b = b0 + j
                        nc.tensor.matmul(
                            out=ppool[:], lhsT=mg3[:, j, :], rhs=h3[:, b, :],
                            start=(b == 0), stop=(b == NB - 1))
                nc.vector.tensor_copy(
                    out=pooled[:].rearrange("p (pb f) -> p pb f", pb=GB)[:, pb, :],
                    in_=ppool[:])
            nc.sync.dma_start(
                out=arin[:].rearrange("(pb g) f -> g pb f", pb=GB),
                in_=pooled[:].rearrange("p (pb f) -> p pb f", pb=GB))
            nc.gpsimd.collective_compute(
                "AllReduce", OP.add, replica_groups=groups,
                ins=[arin[:]], outs=[arout[:]])
            pooled_f = sp.tile([P, GB * H], f32, tag="pooledf")
            nc.sync.dma_start(
                out=pooled_f[:].rearrange("p (pb f) -> p pb f", pb=GB),
                in_=arout[:].rearrange("(pb g) f -> g pb f", pb=GB))
            pf3 = pooled_f[:].rearrange("p (pb f) -> p pb f", pb=GB)

            # ---- readout (fp32)
            outs = sp.tile([P, GB * 16], f32, tag="outs")
            o3 = outs[:].rearrange("p (pb t) -> p pb t", pb=GB)
            for pb in range(GB):
                ptile = pt.tile([H, P], f32, tag="ptr")
                nc.tensor.transpose(out=ptile[:], in_=pf3[:, pb, :],
                                    identity=ident[:])
                lhs = lp.tile([H, P], f32, tag="lhsf")
                nc.vector.tensor_copy(out=lhs[:], in_=ptile[:])
                pmm = pm.tile([P, H], f32, tag="pmm")
                nc.tensor.matmul(out=pmm[:, :16], lhsT=lhs[:], rhs=wout_t[:],
                                 start=True, stop=True)
                nc.vector.tensor_tensor(
                    out=pmm[:, :16], in0=pmm[:, :16], in1=bout_t[:], op=OP.add)
                nc.scalar.activation(out=o3[:, pb, :], in_=pmm[:, :16],
                                     func=AF.Relu)
            nc.sync.dma_start(
                out=out_d[:].rearrange("(pb g) t -> g pb t", pb=GB),
                in_=outs[:].rearrange("p (pb t) -> p pb t", pb=GB))

    nc.compile()
    return nc


# ---------------------------------------------------------------- entry

_CACHE = {}
TRACE = False
LAST_RESULTS = None


def kernel(x, edge_index, batch, W_enc, b_enc, W_conv, b_conv, W_out, b_out,
           num_graphs):
    import ml_dtypes
    cfg = CFG
    N, E, H, L, G, CORES = (cfg[k] for k in ("N", "E", "H", "L", "G", "CORES"))
    P = cfg["P"]
    NLOC, NTOT, PAIRS, WINP, GB = _derived(cfg)

    prep = host_prep(x, edge_index, batch, cfg)
    key = (prep["total_chunks"],
           tuple(tuple(c) for wcalls in prep["schedule"] for c in wcalls))
    if key not in _CACHE:
        from concourse import bass_utils  # noqa: F401
        _CACHE.clear()
        _CACHE[key] = build_bass(prep["schedule"], prep["total_chunks"],
                                 prep["total_idx"], prep["max_nch"], cfg)
    nc = _CACHE[key]

    W_out16 = np.zeros((H, 16), dtype=np.float32)
    W_out16[:, :10] = np.asarray(W_out, dtype=np.float32)
    b_out16 = np.zeros((1, 16), dtype=np.float32)
    b_out16[0, :10] = np.asarray(b_out, dtype=np.float32)
    wconv = np.asarray(W_conv, dtype=np.float32)  # [L, H, H]
    wconv_img = np.ascontiguousarray(
        wconv.transpose(1, 0, 2).reshape(H, L * H)).astype(ml_dtypes.bfloat16)

    shared = {
        "W_enc": np.asarray(W_enc, dtype=np.float32),
        "b_enc": np.tile(np.asarray(b_enc, dtype=np.float32).reshape(1, H), (P, 1)),
        "W_conv": wconv_img,
        "b_conv": np.tile(np.asarray(b_conv, dtype=np.float32).reshape(1, -1), (P, 1)),
        "W_out": W_out16,
        "b_out": np.tile(b_out16, (P, 1)),
        "iota16": np.tile(np.arange(P, dtype=np.int16), (P, 1)),
    }
    in_maps = []
    for c in range(CORES):
        m = dict(shared)
        m["x"] = prep["xs"][c]
        m["ds"] = prep["dss"][c]
        m["bat"] = prep["bats"][c]
        m["idx"] = prep["idx_imgs"][c]
        m["dstrel"] = prep["dstrel_imgs"][c]
        in_maps.append(m)

    from concourse.bass_utils import run_bass_kernel_spmd
    res = run_bass_kernel_spmd(nc, in_maps, core_ids=list(range(CORES)),
                               trace=TRACE)
    global LAST_RESULTS
    LAST_RESULTS = res
    out = res.results[0]["out"]  # [G, 16]
    return np.ascontiguousarray(out[:, :10].astype(np.float32))


# revision 16
# speedup vs baseline: 2.4459x; 1.2270x over previous
"""GCN message-passing kernel for 8 Trainium2 NeuronCores (Bass/Tile).

v3: parity-split half-pair bf16 gather on 4 SWDGE queues.

Algorithm (per core, nodes partitioned across cores):
  h0 = x @ W_enc + b_enc                      (fp32 encoder, bf16 state)
  per conv layer l:
    hsW = (ds * h) @ W_conv[l]                (bf16 blocks on PE)
    AllGather(hsW) -> hsw_full bf16 [NTOT, 64] (+1 pad row pair)
    agg[d] = sum over in-edges of hsw_full[src]:
      dma_gather of 256B rows based at node src (parity-offset base, covers
      src and src+1; only the first 64 cols are used), 4 SWDGE queues
      round-robin, per-chunk selection matmuls reduce on PE into PSUM
    h = silu(ds * (agg + hsW) + b_conv[l])    (fp32 compute, bf16 store)
  pooled = segment_sum(h, batch)              (selection-matmul, bf16)
  AllReduce(pooled); out = relu(pooled @ W_out + b_out)

Host-side prep relabels nodes, partitions edges by destination core, and
groups them by (src window, src parity, dst block).  Group slot count = max
edge count
over cores (exact, not rounded to 128), groups are packed back-to-back into
large gather calls; a 128-slot chunk shared by two groups gets one selection
matrix per (group, chunk) incidence.  Padding slots gather node 0 and carry
dstrel = -1 so is_equal yields a zero M row.
"""

import numpy as np

# ---------------------------------------------------------------- config

CFG = dict(
    N=100000,          # nodes
    E=1600000,         # edges
    H=64,              # hidden
    L=4,               # conv layers
    G=512,             # graphs
    CORES=8,
    P=128,
    NB=98,             # node blocks per core (NLOC = 128*NB)
    NWIN=2,            # gather source windows (WIN/2 pairs <= 32767)
    NQ=4,              # SWDGE queues
    CALL_SLOTS=4096,   # max gather slots per call
)


def _derived(cfg):
    P, NB, CORES, NWIN = cfg["P"], cfg["NB"], cfg["CORES"], cfg["NWIN"]
    NLOC = P * NB
    NTOT = NLOC * CORES
    assert NTOT % NWIN == 0
    WIN = NTOT // NWIN
    assert WIN // 2 <= 32767, "int16 gather index range"
    GB = (cfg["G"] + P - 1) // P
    assert cfg["G"] % P == 0
    return NLOC, NTOT, WIN, GB


# ---------------------------------------------------------------- host prep


def host_prep(x, edge_index, batch, cfg=None):
    """Permute nodes, build per-core inputs + gather schedule.

    Schedule: per window w a list of calls; each call is
      (slots, nch, groups) with groups = [(b, off, g), ...]
    where off is the slot offset of the group inside the call, g its slot
    count (max real edges over cores), slots the 128-padded call total
    (full chunks, so the gather writes every partition) and nch = slots/128
    gather chunks.  Incidences (one selection matrix
    per (group, touched chunk)) are materialized in the dstrel image in
    call/group/chunk order.
    """
    cfg = cfg or CFG
    N, E, H, CORES, P, NB, NWIN = (cfg[k] for k in
                                   ("N", "E", "H", "CORES", "P", "NB", "NWIN"))
    NLOC, NTOT, WIN, GB = _derived(cfg)
    CALL_SLOTS = cfg["CALL_SLOTS"]

    x = np.asarray(x, dtype=np.float32)
    edge_index = np.asarray(edge_index, dtype=np.int64)
    batch = np.asarray(batch, dtype=np.int64)
    src, dst = edge_index[0], edge_index[1]

    deg = np.bincount(dst, minlength=N).astype(np.float64) + 1.0
    ds = (1.0 / np.sqrt(deg)).astype(np.float32)

    # node -> global slot.  Each core gets N//CORES real nodes + dummies.
    per_core = N // CORES
    assert per_core * CORES == N and per_core <= NLOC
    rng = np.random.default_rng(12345)
    order = rng.permutation(N)
    node_slot = np.empty(N, dtype=np.int64)
    for c in range(CORES):
        nodes_c = order[c * per_core:(c + 1) * per_core]
        node_slot[nodes_c] = c * NLOC + np.arange(per_core)

    xs, dss, bats = [], [], []
    for c in range(CORES):
        nodes_c = order[c * per_core:(c + 1) * per_core]
        xl = np.zeros((NLOC, H), dtype=np.float32)
        xl[:per_core] = x[nodes_c]
        dl = np.ones((NLOC,), dtype=np.float32)
        dl[:per_core] = ds[nodes_c]
        bl = np.full((NLOC,), -1, dtype=np.int64)
        bl[:per_core] = batch[nodes_c]
        xs.append(xl)
        dss.append(dl.reshape(P, NB).copy())
        bats.append(bl.reshape(P, NB).astype(np.int16).copy())

    # edges -> (core, window, parity, block, pair idx in window, dst slot)
    s_slot = node_slot[src]
    t_slot = node_slot[dst]
    e_core = t_slot // NLOC
    e_w = s_slot // WIN
    e_par = s_slot % 2
    e_idx = ((s_slot % WIN) // 2).astype(np.int64)
    t_loc = t_slot % NLOC
    e_d = t_loc // NB
    e_b = t_loc % NB

    NSW = NWIN * 2  # super-windows (window, parity)
    e_sw = e_w * 2 + e_par
    key = (e_core * NSW + e_sw) * NB + e_b
    counts = np.bincount(key, minlength=CORES * NSW * NB).reshape(CORES, NSW, NB)
    gsize = counts.max(axis=0)  # [NSW, NB] slots per group (exact)

    # pack groups into calls of <= CALL_SLOTS slots (16-padded)
    schedule = []  # [sw] -> [(slots, nch, [(b, off, g), ...]), ...]
    for sw in range(NSW):
        calls, cur, off = [], [], 0
        for b in range(NB):
            g = int(gsize[sw, b])
            if g == 0:
                continue
            if off + g > CALL_SLOTS and cur:
                slots = (off + P - 1) // P * P
                calls.append((slots, slots // P, cur))
                cur, off = [], 0
            cur.append((b, off, g))
            off += g
        if cur:
            slots = (off + P - 1) // P * P
            calls.append((slots, slots // P, cur))
        schedule.append(calls)

    total_slots = sum(s for sw in range(NSW) for s, _, _ in schedule[sw])
    total_inc = sum(
        (off + g - 1) // P - off // P + 1
        for sw in range(NSW) for _, _, grps in schedule[sw]
        for b, off, g in grps)

    # per-core gather index image + per-incidence dstrel image
    esort = np.lexsort((e_idx, e_b, e_sw, e_core))
    e_idx_s = e_idx[esort]
    e_d_s = e_d[esort]
    grp_key = key[esort]
    grp_starts = np.searchsorted(grp_key, np.arange(CORES * NSW * NB))
    grp_ends = np.searchsorted(grp_key, np.arange(CORES * NSW * NB), side="right")

    idx_imgs, dstrel_imgs = [], []
    for c in range(CORES):
        flat_idx = np.zeros((total_slots,), dtype=np.int16)
        flat_d = np.full((total_inc, P), -1, dtype=np.int16)
        spos = 0  # slot position across calls
        ipos = 0  # incidence position
        for sw in range(NSW):
            for slots, nch, grps in schedule[sw]:
                for b, off, g in grps:
                    gk = (c * NSW + sw) * NB + b
                    s0, s1 = grp_starts[gk], grp_ends[gk]
                    n = s1 - s0
                    assert n <= g
                    flat_idx[spos + off: spos + off + n] = e_idx_s[s0:s1]
                    k0, k1 = off // P, (off + g - 1) // P
                    for k in range(k0, k1 + 1):
                        # slots of this group inside chunk k (call-local)
                        t0 = max(off, k * P)
                        t1 = min(off + n, (k + 1) * P)
                        if t1 > t0:
                            flat_d[ipos, t0 - k * P: t1 - k * P] = \
                                e_d_s[s0 + (t0 - off): s0 + (t1 - off)]
                        ipos += 1
                spos += slots
        assert spos == total_slots and ipos == total_inc
        img = flat_idx.reshape(total_slots // 16, 16).T
        img = np.tile(img, (P // 16, 1)).copy()
        idx_imgs.append(img)
        dstrel_imgs.append(np.ascontiguousarray(flat_d.T))  # [P, total_inc]

    return dict(
        xs=xs, dss=dss, bats=bats, idx_imgs=idx_imgs, dstrel_imgs=dstrel_imgs,
        schedule=schedule, total_slots=total_slots, total_inc=total_inc,
    )


# ---------------------------------------------------------------- bass build


def build_bass(schedule, total_slots, total_inc, cfg=None):
    import concourse.bacc as bacc
    import concourse.tile as tile
    from concourse import mybir
    from concourse.masks import make_identity

    cfg = cfg or CFG
    N, E, H, L, G, CORES, P, NB, NWIN, NQ = (cfg[k] for k in
                                             ("N", "E", "H", "L", "G", "CORES",
                                              "P", "NB", "NWIN", "NQ"))
    NLOC, NTOT, WIN, GB = _derived(cfg)
    MAXCH = (cfg["CALL_SLOTS"] + P - 1) // P
    f32, bf16, i16 = mybir.dt.float32, mybir.dt.bfloat16, mybir.dt.int16
    AF = mybir.ActivationFunctionType
    OP = mybir.AluOpType

    from concourse.bass import AP

    def mid_bcast(ap2d, nmid):
        return AP(ap2d.tensor, ap2d.offset,
                  [list(ap2d.ap[0]), [0, nmid], list(ap2d.ap[1])])

    nc = bacc.Bacc("TRN2", target_bir_lowering=False, debug=False,
                   num_devices=CORES, num_swdge_queues=NQ)
    groups = [list(range(CORES))]

    # ---- I/O
    x_d = nc.dram_tensor("x", [NLOC, H], f32, kind="ExternalInput").ap()
    ds_d = nc.dram_tensor("ds", [P, NB], f32, kind="ExternalInput").ap()
    bat_d = nc.dram_tensor("bat", [P, NB], i16, kind="ExternalInput").ap()
    idx_d = nc.dram_tensor("idx", [P, total_slots // 16], i16,
                           kind="ExternalInput").ap()
    dstrel_d = nc.dram_tensor("dstrel", [P, total_inc], i16,
                              kind="ExternalInput").ap()
    wenc_d = nc.dram_tensor("W_enc", [H, H], f32, kind="ExternalInput").ap()
    benc_d = nc.dram_tensor("b_enc", [P, H], f32, kind="ExternalInput").ap()
    wconv_d = nc.dram_tensor("W_conv", [H, L * H], bf16,
                             kind="ExternalInput").ap()
    bconv_d = nc.dram_tensor("b_conv", [P, L * H], f32, kind="ExternalInput").ap()
    wout_d = nc.dram_tensor("W_out", [H, 16], f32, kind="ExternalInput").ap()
    bout_d = nc.dram_tensor("b_out", [P, 16], f32, kind="ExternalInput").ap()
    iota16_d = nc.dram_tensor("iota16", [P, P], i16, kind="ExternalInput").ap()
    out_d = nc.dram_tensor("out", [G, 16], f32, kind="ExternalOutput").ap()

    # ---- internal DRAM (pair-major bf16 gather table, +1 pad pair row)
    agin = nc.dram_tensor("agin", [NLOC // 2, P], bf16).ap()
    hsw_full = nc.dram_tensor("hsw_full", [NTOT // 2 + 1, P], bf16,
                              addr_space="Shared").ap()
    arin = nc.dram_tensor("arin", [G, H], f32).ap()
    arout = nc.dram_tensor("arout", [G, H], f32).ap()

    with tile.TileContext(nc) as tc:
        with (
            tc.tile_pool(name="const", bufs=1) as cp,
            tc.tile_pool(name="state", bufs=1) as sp,
            tc.tile_pool(name="msg", bufs=6) as mp,
            tc.tile_pool(name="mb", bufs=4) as mbp,
            tc.tile_pool(name="mgp", bufs=2) as mgp,
            tc.tile_pool(name="lhs", bufs=3) as lp,
            tc.tile_pool(name="pa", bufs=3, space="PSUM") as pa,
            tc.tile_pool(name="pt", bufs=1, space="PSUM") as pt,
            tc.tile_pool(name="ptb", bufs=1, space="PSUM") as ptb,
            tc.tile_pool(name="pm", bufs=2, space="PSUM") as pm,
        ):
            # ---- constants to SBUF
            ident = cp.tile([P, P], f32)
            make_identity(nc, ident[:])
            ident_bf = cp.tile([P, P], bf16)
            nc.vector.tensor_copy(out=ident_bf[:], in_=ident[:])
            iota16_t = cp.tile([P, P], i16)
            nc.sync.dma_start(out=iota16_t[:], in_=iota16_d[:])
            wenc_t = cp.tile([H, H], f32)
            nc.sync.dma_start(out=wenc_t[:], in_=wenc_d[:])
            benc_t = cp.tile([P, H], f32)
            nc.sync.dma_start(out=benc_t[:], in_=benc_d[:])
            wconv_t = cp.tile([H, L * H], bf16)
            nc.sync.dma_start(out=wconv_t[:], in_=wconv_d[:])
            bconv_t = cp.tile([P, L * H], f32)
            nc.sync.dma_start(out=bconv_t[:], in_=bconv_d[:])
            wout_t = cp.tile([H, 16], f32)
            nc.sync.dma_start(out=wout_t[:], in_=wout_d[:])
            bout_t = cp.tile([P, 16], f32)
            nc.sync.dma_start(out=bout_t[:], in_=bout_d[:])
            ds_t = cp.tile([P, NB], f32)
            nc.sync.dma_start(out=ds_t[:], in_=ds_d[:])
            bat_t = cp.tile([P, NB], i16)
            nc.sync.dma_start(out=bat_t[:], in_=bat_d[:])
            idx_t = cp.tile([P, total_slots // 16], i16)
            nc.sync.dma_start(out=idx_t[:], in_=idx_d[:])
            dstrel_t = cp.tile([P, total_inc], i16)
            nc.sync.dma_start(out=dstrel_t[:], in_=dstrel_d[:])

            # ---- state tiles (layout [P(d), NB, H], local node = d*NB+b)
            h_t = sp.tile([P, NB * H], bf16, tag="h")
            hsw_t = sp.tile([P, NB * H], bf16, tag="hsw")
            agg_t = sp.tile([P, NB * H], f32, tag="agg")
            h3 = h_t[:].rearrange("p (b f) -> p b f", b=NB)
            hsw3 = hsw_t[:].rearrange("p (b f) -> p b f", b=NB)
            agg3 = agg_t[:].rearrange("p (b f) -> p b f", b=NB)

            # ---- prelude: h0 = x @ W_enc + b_enc (fp32, store bf16)
            nc.sync.dma_start(
                out=agg_t[:],
                in_=x_d[:].rearrange("(d b) f -> d (b f)", d=P))
            x3 = agg3
            for b in range(NB):
                ptile = pt.tile([H, P], f32, tag="ptr")
                nc.tensor.transpose(out=ptile[:], in_=x3[:, b, :],
                                    identity=ident[:])
                lhs = lp.tile([H, P], f32, tag="lhsf")
                nc.vector.tensor_copy(out=lhs[:], in_=ptile[:])
                pmm = pm.tile([P, H], f32, tag="pmm")
                nc.tensor.matmul(out=pmm[:], lhsT=lhs[:], rhs=wenc_t[:],
                                 start=True, stop=True)
                nc.vector.tensor_tensor(
                    out=h3[:, b, :], in0=pmm[:], in1=benc_t[:], op=OP.add)

            # ---- conv layers
            for l in range(L):
                # hs = h * ds  (in place, bf16)
                nc.vector.tensor_tensor(
                    out=h3, in0=h3,
                    in1=ds_t[:].to_broadcast([P, NB, H]),
                    op=OP.mult)
                # hsW = hs @ W_conv[l]  (bf16)
                wl = wconv_t[:].rearrange("i (l o) -> l i o", l=L)[l]
                for b in range(NB):
                    ptile = ptb.tile([H, P], bf16, tag="ptrb")
                    nc.tensor.transpose(out=ptile[:], in_=h3[:, b, :],
                                        identity=ident_bf[:])
                    lhs = lp.tile([H, P], bf16, tag="lhsb")
                    nc.vector.tensor_copy(out=lhs[:], in_=ptile[:])
                    pmm = pm.tile([P, H], f32, tag="pmm")
                    nc.tensor.matmul(out=pmm[:], lhsT=lhs[:], rhs=wl,
                                     start=True, stop=True)
                    nc.vector.tensor_copy(out=hsw3[:, b, :], in_=pmm[:])
                # ship local hsW (pair-major), AllGather into hsw_full
                nc.sync.dma_start(
                    out=agin[:].rearrange("(d bb) tf -> d (bb tf)", d=P),
                    in_=hsw_t[:])
                nc.gpsimd.collective_compute(
                    "AllGather", OP.bypass, replica_groups=groups,
                    ins=[agin[:]], outs=[hsw_full[0:NTOT // 2, :]])

                # aggregate.  Gather reads 256B at node row n of a
                # [NTOT, 64]-shaped bf16 view (covers rows n, n+1); only
                # cols 0:64 of each msg slot are consumed.
                nc.vector.memset(agg_t[:], 0.0)
                slot_off = 0
                inc_off = 0
                call_no = 0
                for sw in range(NWIN * 2):
                    w, par = sw // 2, sw % 2
                    # 256B-stride rows starting at node w*WIN + par:
                    # pair j of super-window = nodes (w*WIN + par + 2j, +1);
                    # only the first 64 cols (the src node) are consumed.
                    win_ap = AP(hsw_full.tensor,
                                hsw_full.offset + (w * WIN + par) * H,
                                [[P, WIN // 2], [1, P]])
                    for slots, nch, grps in schedule[sw]:
                        msg = mp.tile([P, MAXCH * P], bf16, tag="msg")
                        nc.gpsimd.dma_gather(
                            out_ap=msg[:, :nch * P]
                                .rearrange("p (s f) -> p s f", f=P),
                            in_ap=win_ap,
                            idxs_ap=idx_t[:, slot_off // 16:
                                          (slot_off + slots) // 16],
                            num_idxs=slots,
                            num_idxs_reg=slots,
                            elem_size=P,
                            single_packet=False,
                            queue_num=call_no % NQ,
                        )
                        ninc = sum((off + g - 1) // P - off // P + 1
                                   for _, off, g in grps)
                        mtile = mbp.tile([P, (MAXCH + 8) * P], bf16,
                                         tag="mtile")
                        nc.vector.tensor_tensor(
                            out=mtile[:, :ninc * P]
                                .rearrange("p (c q) -> p c q", q=P),
                            in0=dstrel_t[:, inc_off:inc_off + ninc]
                                .to_broadcast([P, ninc, P]),
                            in1=mid_bcast(iota16_t[:], ninc),
                            op=OP.is_equal)
                        msg3 = msg[:].rearrange("p (s f) -> p s f", f=P)
                        mt3 = mtile[:].rearrange("p (c q) -> p c q", q=P)
                        j = 0
                        for b, off, g in grps:
                            k0, k1 = off // P, (off + g - 1) // P
                            pagg = pa.tile([P, H], f32, tag="pagg")
                            for k in range(k0, k1 + 1):
                                nc.tensor.matmul(
                                    out=pagg[:], lhsT=mt3[:, j, :],
                                    rhs=msg3[:, k, 0:H],
                                    start=(k == k0), stop=(k == k1))
                                j += 1
                            nc.vector.tensor_tensor(
                                out=agg3[:, b, :], in0=agg3[:, b, :],
                                in1=pagg[:], op=OP.add)
                        assert j == ninc
                        slot_off += slots
                        inc_off += ninc
                        call_no += 1
                assert slot_off == total_slots and inc_off == total_inc

                # h = silu(ds * (agg + hsW) + b_conv[l])
                nc.vector.tensor_tensor(out=agg_t[:], in0=agg_t[:],
                                        in1=hsw_t[:], op=OP.add)
                nc.vector.tensor_tensor(
                    out=agg3, in0=agg3,
                    in1=ds_t[:].to_broadcast([P, NB, H]),
                    op=OP.mult)
                nc.vector.tensor_tensor(
                    out=agg3, in0=agg3,
                    in1=mid_bcast(bconv_t[:, l * H:(l + 1) * H], NB),
                    op=OP.add)
                nc.scalar.activation(out=hsw_t[:], in_=agg_t[:],
                                     func=AF.Sigmoid)
                nc.vector.tensor_tensor(out=h_t[:], in0=hsw_t[:],
                                        in1=agg_t[:], op=OP.mult)

            # ---- pooling: pooled[pb*P+g] = sum_{nodes with bat==pb*P+g} h
            pooled = sp.tile([P, GB * H], f32, tag="pooled")
            MG = 14  # blocks per Mp build
            for pb in range(GB):
                ppool = pa.tile([P, H], f32, tag="pagg")
                for b0 in range(0, NB, MG):
                    nbk = min(MG, NB - b0)
                    mg = mgp.tile([P, MG * P], bf16, tag="mg")
                    mgi = mgp.tile([P, MG * P], i16, tag="mgi")
                    nc.vector.tensor_scalar(
                        out=mgi[:, :nbk * P]
                            .rearrange("p (c q) -> p c q", q=P),
                        in0=bat_t[:, b0:b0 + nbk]
                            .to_broadcast([P, nbk, P]),
                        scalar1=pb * P,
                        scalar2=None,
                        op0=OP.subtract,
                    )
                    nc.vector.tensor_tensor(
                        out=mg[:, :nbk * P]
                            .rearrange("p (c q) -> p c q", q=P),
                        in0=mgi[:, :nbk * P]
                            .rearrange("p (c q) -> p c q", q=P),
                        in1=mid_bcast(iota16_t[:], nbk),
                        op=OP.is_equal)
                    mg3 = mg[:].rearrange("p (c q) -> p c q", q=P)
                    for j in range(nbk):
                        b = b0 + j
                        nc.tensor.matmul(
                            out=ppool[:], lhsT=mg3[:, j, :], rhs=h3[:, b, :],
                            start=(b == 0), stop=(b == NB - 1))
                nc.vector.tensor_copy(
                    out=pooled[:].rearrange("p (pb f) -> p pb f", pb=GB)[:, pb, :],
                    in_=ppool[:])
            nc.sync.dma_start(
                out=arin[:].rearrange("(pb g) f -> g pb f", pb=GB),
                in_=pooled[:].rearrange("p (pb f) -> p pb f", pb=GB))
            nc.gpsimd.collective_compute(
                "AllReduce", OP.add, replica_groups=groups,
                ins=[arin[:]], outs=[arout[:]])
            pooled_f = sp.tile([P, GB * H], f32, tag="pooledf")
            nc.sync.dma_start(
                out=pooled_f[:].rearrange("p (pb f) -> p pb f", pb=GB),
                in_=arout[:].rearrange("(pb g) f -> g pb f", pb=GB))
            pf3 = pooled_f[:].rearrange("p (pb f) -> p pb f", pb=GB)

            # ---- readout (fp32)
            outs = sp.tile([P, GB * 16], f32, tag="outs")
            o3 = outs[:].rearrange("p (pb t) -> p pb t", pb=GB)
            for pb in range(GB):
                ptile = pt.tile([H, P], f32, tag="ptr")
                nc.tensor.transpose(out=ptile[:], in_=pf3[:, pb, :],
                                    identity=ident[:])
                lhs = lp.tile([H, P], f32, tag="lhsf")
                nc.vector.tensor_copy(out=lhs[:], in_=ptile[:])
                pmm = pm.tile([P, H], f32, tag="pmm")
                nc.tensor.matmul(out=pmm[:, :16], lhsT=lhs[:], rhs=wout_t[:],
                                 start=True, stop=True)
                nc.vector.tensor_tensor(
                    out=pmm[:, :16], in0=pmm[:, :16], in1=bout_t[:], op=OP.add)
                nc.scalar.activation(out=o3[:, pb, :], in_=pmm[:, :16],
                                     func=AF.Relu)
            nc.sync.dma_start(
                out=out_d[:].rearrange("(pb g) t -> g pb t", pb=GB),
                in_=outs[:].rearrange("p (pb t) -> p pb t", pb=GB))

    nc.compile()
    return nc


# ---------------------------------------------------------------- entry

_CACHE = {}
TRACE = False
LAST_RESULTS = None


def kernel(x, edge_index, batch, W_enc, b_enc, W_conv, b_conv, W_out, b_out,
           num_graphs):
    import ml_dtypes
    cfg = CFG
    N, E, H, L, G, CORES = (cfg[k] for k in ("N", "E", "H", "L", "G", "CORES"))
    P = cfg["P"]
    NLOC, NTOT, WIN, GB = _derived(cfg)

    prep = host_prep(x, edge_index, batch, cfg)
    key = (prep["total_slots"], prep["total_inc"],
           tuple((s, n, tuple(g)) for wcalls in prep["schedule"]
                 for s, n, gr in wcalls for g in gr))
    if key not in _CACHE:
        from concourse import bass_utils  # noqa: F401
        _CACHE.clear()
        _CACHE[key] = build_bass(prep["schedule"], prep["total_slots"],
                                 prep["total_inc"], cfg)
    nc = _CACHE[key]

    W_out16 = np.zeros((H, 16), dtype=np.float32)
    W_out16[:, :10] = np.asarray(W_out, dtype=np.float32)
    b_out16 = np.zeros((1, 16), dtype=np.float32)
    b_out16[0, :10] = np.asarray(b_out, dtype=np.float32)
    wconv = np.asarray(W_conv, dtype=np.float32)  # [L, H, H]
    wconv_img = np.ascontiguousarray(
        wconv.transpose(1, 0, 2).reshape(H, L * H)).astype(ml_dtypes.bfloat16)

    shared = {
        "W_enc": np.asarray(W_enc, dtype=np.float32),
        "b_enc": np.tile(np.asarray(b_enc, dtype=np.float32).reshape(1, H), (P, 1)),
        "W_conv": wconv_img,
        "b_conv": np.tile(np.asarray(b_conv, dtype=np.float32).reshape(1, -1), (P, 1)),
        "W_out": W_out16,
        "b_out": np.tile(b_out16, (P, 1)),
        "iota16": np.tile(np.arange(P, dtype=np.int16), (P, 1)),
    }
    in_maps = []
    for c in range(CORES):
        m = dict(shared)
        m["x"] = prep["xs"][c]
        m["ds"] = prep["dss"][c]
        m["bat"] = prep["bats"][c]
        m["idx"] = prep["idx_imgs"][c]
        m["dstrel"] = prep["dstrel_imgs"][c]
        in_maps.append(m)

    from concourse.bass_utils import run_bass_kernel_spmd
    res = run_bass_kernel_spmd(nc, in_maps, core_ids=list(range(CORES)),
                               trace=TRACE)
    global LAST_RESULTS
    LAST_RESULTS = res
    out = res.results[0]["out"]  # [G, 16]
    return np.ascontiguousarray(out[:, :10].astype(np.float32))
